# Initial kernel scaffold
#
"""Trainium2 Bass kernel for nn_Despawn2D (8-level row-wise DWT fwd+inv).

Self-contained: builds per-level banded operator matrices from the wavelet on
the host (cheap, data-independent), then runs the whole transform as TensorE
matmuls on 8 NeuronCores, data-parallel over rows (512 rows/core).

Layouts:
  - intermediates column-major: tiles [<=128 cols (partitions), 512 rows (free)]
  - fwd/inv levels: operator-stationary matmuls, out_block += mat.T @ in_block
  - final rec + coeffs: produced row-major via data-stationary matmuls
    (stationary = data block, moving = narrow banded operator)
"""
import numpy as np
from collections import defaultdict

NCOLS = 4096
NROWS = 4096
LEVELS = 8
BW = 128
NCORES = 8
ROWS_PER_CORE = NROWS // NCORES  # 512
RCHUNKS = ROWS_PER_CORE // BW    # 4
USE_F32R = False

# ------------------------------------------------------------------ plan

def tile_width(name):
    if name == "x":
        return NCOLS
    if name[0] in "da":
        return NCOLS >> (int(name[1]) + 1)
    if name == "c5":
        return 128
    if name == "c6":
        return 64
    if name == "c7":
        return 32
    if name.startswith("rec"):
        return NCOLS >> int(name[3])
    raise KeyError(name)


def coeff_col_off(l):
    return sum(NCOLS >> (i + 1) for i in range(l))


def _filters(w):
    alt = (-1.0) ** np.arange(16)
    g = w[::-1] * alt
    return w[0::2], w[1::2], g[0::2], g[1::2], w[::-1], w * alt


def _fwd_input_view(l):
    if l == 0:
        return "x", 0
    if 1 <= l <= 5:
        return f"a{l-1}", 0
    if l == 6:
        return "c5", 64
    return "c6", 32


def _fwd_out_spec(l, t, part):
    if l <= 4:
        return (f"d{l}", t), (f"a{l}", t)
    name = {5: "c5", 6: "c6", 7: "c7"}[l]
    return (name, t), (name, part + t)


def _circular_span(cols, n):
    cols = np.sort(cols)
    gaps = np.diff(np.concatenate([cols, [cols[0] + n]]))
    gi = int(np.argmax(gaps))
    return int(cols[(gi + 1) % cols.size]), int(n - gaps[gi] + 1)


def build_plan(wavelet):
    """Structure is wavelet-independent (uses a structural mask); values from
    `wavelet`. Returns dict of matmul descriptors."""
    wavelet = np.asarray(wavelet, dtype=np.float64)
    wstruct = np.arange(1.0, 17.0)[None, :] * (1.0 + np.arange(LEVELS)[:, None])
    plan = {"fwd": [], "inv": [], "inv0": [], "extract": []}

    def fwd_level(l):
        we, wo, ge, go, _, _ = _filters(wavelet[l])
        s_we, s_wo, s_ge, s_go, _, _ = _filters(wstruct[l])
        L = NCOLS >> l
        part = L // 2
        in_tile, in_off = _fwd_input_view(l)
        acc = defaultdict(lambda: defaultdict(lambda: np.zeros((2, BW, BW))))
        for t in range(part):
            (dt_t, dt_c), (at_t, at_c) = _fwd_out_spec(l, t, part)
            for k in range(8):
                s = (t - k) % part
                for j, cd, ca, sd, sa in ((2 * s, we[k], ge[k], s_we[k], s_ge[k]),
                                          (2 * s + 1, wo[k], go[k], s_wo[k], s_go[k])):
                    p = in_off + j
                    ib, ir = p // BW, p % BW
                    a1 = acc[(dt_t, dt_c // BW)][ib]
                    a1[0, ir, dt_c % BW] += cd
                    a1[1, ir, dt_c % BW] += sd
                    a2 = acc[(at_t, at_c // BW)][ib]
                    a2[0, ir, at_c % BW] += ca
                    a2[1, ir, at_c % BW] += sa
        return _emit(acc, in_tile)

    for l in range(LEVELS):
        plan["fwd"].append(fwd_level(l))

    for l in range(LEVELS - 1, 0, -1):
        _, _, _, _, ws, ss = _filters(wavelet[l])
        _, _, _, _, s_ws, s_ss = _filters(wstruct[l])
        P = NCOLS >> (l + 1)
        n2 = 2 * P
        d_tile = f"d{l}" if l <= 4 else {5: "c5", 6: "c6", 7: "c7"}[l]
        r_tile, r_off = ("c7", 16) if l == 7 else (f"rec{l+1}", 0)
        out_tile = f"rec{l}"
        accs = {d_tile: defaultdict(lambda: defaultdict(lambda: np.zeros((2, BW, BW)))),
                r_tile: defaultdict(lambda: defaultdict(lambda: np.zeros((2, BW, BW))))}
        for t in range(n2):
            ob, oc = t // BW, t % BW
            for m in range(16):
                u = (t - m) % n2
                if u % 2 == 0:
                    s = u // 2
                    ad = accs[d_tile][(out_tile, ob)][s // BW]
                    ad[0, s % BW, oc] += ws[m]
                    ad[1, s % BW, oc] += s_ws[m]
                    pr = r_off + s
                    ar = accs[r_tile][(out_tile, ob)][pr // BW]
                    ar[0, pr % BW, oc] += ss[m]
                    ar[1, pr % BW, oc] += s_ss[m]
        descs = []
        for tname, a in accs.items():
            descs += _emit(a, tname)
        plan["inv"].append((l, descs))

    # inverse level 0 -> row-major rec via data-stationary matmuls
    _, _, _, _, ws, ss = _filters(wavelet[0])
    _, _, _, _, s_ws, s_ss = _filters(wstruct[0])
    P = NCOLS // 2
    mov = {"d0": defaultdict(lambda: np.zeros((2, BW, NCOLS))),
           "rec1": defaultdict(lambda: np.zeros((2, BW, NCOLS)))}
    for t in range(NCOLS):
        for m in range(16):
            u = (t - m) % NCOLS
            if u % 2 == 0:
                s = u // 2
                md = mov["d0"][s // BW]
                md[0, s % BW, t] += ws[m]
                md[1, s % BW, t] += s_ws[m]
                mr = mov["rec1"][s // BW]
                mr[0, s % BW, t] += ss[m]
                mr[1, s % BW, t] += s_ss[m]
    CH = 512
    for tname in ("d0", "rec1"):
        for blk in range(P // BW):
            full = mov[tname][blk]
            cols = np.nonzero(np.any(full[1] != 0.0, axis=0))[0]
            lo, width = _circular_span(cols, NCOLS)
            # split circular span at CH boundaries of global col index
            pos = 0
            while pos < width:
                g = (lo + pos) % NCOLS
                take = min(width - pos, CH - (g % CH))
                idx = (g + np.arange(take)) % NCOLS
                sub = full[0][:, idx]
                subs = full[1][:, idx]
                rows = np.nonzero(np.any(subs != 0.0, axis=1))[0]
                if rows.min() >= 96:
                    plan["inv0"].append((tname, blk, 96, 32, sub[96:128].copy(),
                                         g // CH, g % CH, take))
                else:
                    plan["inv0"].append((tname, blk, 0, BW, sub.copy(),
                                         g // CH, g % CH, take))
                pos += take

    # extraction -> row-major coeffs
    ext = plan["extract"]
    for l in range(5):
        width = NCOLS >> (l + 1)
        off = coeff_col_off(l)
        for blk in range(width // BW):
            ext.append((f"d{l}", blk, BW, np.eye(BW), off + BW * blk, BW))
    sel = np.zeros((128, 64)); sel[:64, :] = np.eye(64)
    ext.append(("c5", 0, 128, sel, coeff_col_off(5), 64))
    sel6 = np.zeros((64, 32)); sel6[:32, :] = np.eye(32)
    ext.append(("c6", 0, 64, sel6, coeff_col_off(6), 32))
    ext.append(("c7", 0, 32, np.eye(32), coeff_col_off(7), 32))
    return plan


def _emit(acc, in_tile):
    descs = []
    inw = tile_width(in_tile)
    for (out_tile, ob), inblocks in sorted(acc.items()):
        obw = min(BW, tile_width(out_tile))
        for ib, mat in sorted(inblocks.items()):
            ibw = min(BW, inw - ib * BW)
            m, ms = mat[0, :ibw, :obw], mat[1, :ibw, :obw]
            rows = np.nonzero(np.any(ms != 0.0, axis=1))[0]
            if rows.size == 0:
                continue
            if ibw == BW and rows.min() >= 96:
                descs.append((out_tile, ob, obw, in_tile, ib, 96, 32, m[96:128].copy()))
            else:
                descs.append((out_tile, ob, obw, in_tile, ib, 0, ibw, m.copy()))
    return descs


# ------------------------------------------------------------- bass build
_CACHE = {}


def _build_bass(plan_struct):
    import concourse.bacc as bacc
    import concourse.mybir as mybir
    from concourse.tile import TileContext

    F32 = mybir.dt.float32
    DT = mybir.dt.float32r if USE_F32R else F32

    # ---- pack unique stationary matrices (col-major ops) into one array
    mat_off = {}   # id(desc mat placement) by content
    packs = []     # (koff, K, M, values)
    totM = 0

    def pack_mat(koff, K, mat):
        key = (koff, K, mat.shape[1], mat.tobytes())
        if key in mat_off:
            return mat_off[key]
        nonlocal totM
        off = totM
        totM += mat.shape[1]
        mat_off[key] = off
        packs.append((koff, K, mat))
        return off

    col_descs = []  # (out_tile, ob, obw, in_tile, ib, koff, K, matoff, M)
    for l in range(LEVELS):
        for (ot, ob, obw, it, ib, koff, K, mat) in plan_struct["fwd"][l]:
            col_descs.append(("fwd", l, ot, ob, obw, it, ib, koff, K,
                              pack_mat(koff, K, mat), mat.shape[1]))
    for l, descs in plan_struct["inv"]:
        for (ot, ob, obw, it, ib, koff, K, mat) in descs:
            col_descs.append(("inv", l, ot, ob, obw, it, ib, koff, K,
                              pack_mat(koff, K, mat), mat.shape[1]))

    # ---- pack moving matrices (inv0 + extraction)
    mov_off = {}
    mpacks = []
    totN = 0

    def pack_mov(koff, K, mat):
        key = (koff, K, mat.shape[1], mat.tobytes())
        if key in mov_off:
            return mov_off[key]
        nonlocal totN
        off = totN
        totN += mat.shape[1]
        mov_off[key] = off
        mpacks.append((koff, K, mat))
        return off

    inv0_descs = []
    for (st, blk, koff, K, mat, ch, noff, N) in plan_struct["inv0"]:
        inv0_descs.append((st, blk, koff, K, pack_mov(koff, K, mat), ch, noff, N))
    ext_descs = []
    for (st, blk, K, mat, coff, N) in plan_struct["extract"]:
        ext_descs.append((st, blk, K, pack_mov(0, K, mat), coff, N))

    nc = bacc.Bacc(None, target_bir_lowering=False)
    xin = nc.declare_dram_parameter("xin", [ROWS_PER_CORE, NCOLS], DT, isOutput=False)
    mats_d = nc.declare_dram_parameter("mats", [128, totM], DT, isOutput=False)
    movs_d = nc.declare_dram_parameter("movs", [128, totN], DT, isOutput=False)
    ident_d = nc.declare_dram_parameter("ident", [128, 128], DT, isOutput=False)
    rec_out = nc.declare_dram_parameter("rec_out", [ROWS_PER_CORE, NCOLS], F32, isOutput=True)
    coef_out = nc.declare_dram_parameter("coef_out", [ROWS_PER_CORE, NCOLS], F32, isOutput=True)

    evac_ctr = [0]

    with TileContext(nc) as tc:
        with tc.tile_pool(name="const", bufs=1) as cpool, \
             tc.tile_pool(name="xrm", bufs=2) as xrm_pool, \
             tc.tile_pool(name="xcm", bufs=34) as xcm_pool, \
             tc.tile_pool(name="d0p", bufs=16) as d0_pool, \
             tc.tile_pool(name="blk", bufs=30) as blk_pool, \
             tc.tile_pool(name="stg", bufs=2) as stg_pool, \
             tc.tile_pool(name="ps", bufs=1, space="PSUM") as psp:

            mats_sb = cpool.tile([128, totM], DT, tag="mats")
            movs_sb = cpool.tile([128, totN], DT, tag="movs")
            ident_sb = cpool.tile([128, 128], DT, tag="ident")
            nc.sync.dma_start(out=mats_sb[:], in_=mats_d[:])
            nc.sync.dma_start(out=movs_sb[:], in_=movs_d[:])
            nc.sync.dma_start(out=ident_sb[:], in_=ident_d[:])

            def evac(dst_ap, src_ap):
                if evac_ctr[0] % 2 == 0:
                    nc.scalar.copy(dst_ap, src_ap)
                else:
                    nc.vector.tensor_copy(dst_ap, src_ap)
                evac_ctr[0] += 1

            tiles = {}

            def get_tile(name, blk):
                key = (name, blk)
                if key not in tiles:
                    w = min(BW, tile_width(name))
                    if name == "x":
                        t = xcm_pool.tile([BW, ROWS_PER_CORE], DT, tag="xcm")
                    elif name == "a0":
                        t = xcm_pool.tile([BW, ROWS_PER_CORE], DT, tag="xcm")
                    elif name == "d0":
                        t = d0_pool.tile([BW, ROWS_PER_CORE], DT, tag="d0")
                    else:
                        t = blk_pool.tile([BW, ROWS_PER_CORE], DT, tag="blk")
                    tiles[key] = t
                return tiles[key]

            # ---- input transposes (row-chunk major) ----
            psum_ctr = [0]

            def psum_tile(shape, nbanks=1):
                t = psp.tile(shape, F32, tag=f"pp{psum_ctr[0] % 6}")
                psum_ctr[0] += 1
                return t

            for r in range(RCHUNKS):
                for h in range(2):
                    xr = xrm_pool.tile([BW, NCOLS // 2], DT, tag="xrm")
                    nc.sync.dma_start(out=xr[:], in_=xin[BW * r:BW * (r + 1),
                                                         2048 * h:2048 * (h + 1)])
                    for bb in range(16):
                        b = 16 * h + bb
                        pt = psum_tile([128, 128])
                        nc.tensor.matmul(pt[:], xr[:, BW * bb:BW * (bb + 1)],
                                         ident_sb[:], start=True, stop=True)
                        xt = get_tile("x", b)
                        evac(xt[:, BW * r:BW * (r + 1)], pt[:])

            # ---- column-major levels (fwd then inv 7..1) ----
            groups = defaultdict(list)
            order = []
            for d in col_descs:
                key = (d[0], d[1], d[2], d[3])
                if key not in groups:
                    order.append(key)
                groups[key].append(d)
            for key in order:
                descs = groups[key]
                _, _, ot, ob = key
                obw = descs[0][4]
                pt = psum_tile([obw, ROWS_PER_CORE])
                for i, (_, _, _, _, _, it, ib, koff, K, moff, M) in enumerate(descs):
                    lhsT = mats_sb[koff:koff + K, moff:moff + M]
                    rhs = get_tile(it, ib)[koff:koff + K, :]
                    tp = (96, 0) if koff == 96 else None
                    nc.tensor.matmul(pt[0:M, :], lhsT, rhs, start=(i == 0),
                                     stop=(i == len(descs) - 1), tile_position=tp)
                out_t = get_tile(ot, ob)
                evac(out_t[0:obw, :], pt[0:obw, :])

            # ---- extraction (coeffs, row-major) ----
            # group pieces by (rowchunk, kb column block of 1024)
            ext_by_q = defaultdict(list)
            for (st, blk, K, moff, coff, N) in ext_descs:
                ext_by_q[coff // 1024].append((st, blk, K, moff, coff, N))
            for r in range(RCHUNKS):
                for q in sorted(ext_by_q):
                    stg = stg_pool.tile([BW, 1024], F32, tag="cstg")
                    for (st, blk, K, moff, coff, N) in ext_by_q[q]:
                        pt = psum_tile([128, N])
                        lhsT = get_tile(st, blk)[0:K, BW * r:BW * (r + 1)]
                        rhs = movs_sb[0:K, moff:moff + N]
                        nc.tensor.matmul(pt[:, 0:N], lhsT, rhs, start=True, stop=True)
                        evac(stg[:, coff - 1024 * q:coff - 1024 * q + N], pt[:, 0:N])
                    nc.sync.dma_start(
                        out=coef_out[BW * r:BW * (r + 1), 1024 * q:1024 * (q + 1)],
                        in_=stg[:])

            # ---- inv0 (rec, row-major) ----
            by_chunk = defaultdict(list)
            for (st, blk, koff, K, moff, ch, noff, N) in inv0_descs:
                by_chunk[ch].append((st, blk, koff, K, moff, noff, N))
            for r in range(RCHUNKS):
                for q in range(2):  # two 1024-col groups per staging DMA... 4 per row
                    pass
                stg = None
                for ch in range(NCOLS // 512):
                    if ch % 2 == 0:
                        stg = stg_pool.tile([BW, 1024], F32, tag="rstg")
                    pt = psum_tile([128, 512])
                    dl = by_chunk[ch]
                    for i, (st, blk, koff, K, moff, noff, N) in enumerate(dl):
                        lhsT = get_tile(st, blk)[koff:koff + K, BW * r:BW * (r + 1)]
                        rhs = movs_sb[koff:koff + K, moff:moff + N]
                        tp = (96, 0) if koff == 96 else None
                        nc.tensor.matmul(pt[:, noff:noff + N], lhsT, rhs,
                                         start=(i == 0), stop=(i == len(dl) - 1),
                                         tile_position=tp)
                    evac(stg[:, 512 * (ch % 2):512 * (ch % 2) + 512], pt[:])
                    if ch % 2 == 1:
                        nc.sync.dma_start(
                            out=rec_out[BW * r:BW * (r + 1),
                                        1024 * (ch // 2):1024 * (ch // 2 + 1)],
                            in_=stg[:])

    nc.compile()

    # host-side packed arrays (values filled per wavelet by caller)
    return nc, packs, mpacks, totM, totN


def _pack_values(packs, totM):
    arr = np.zeros((128, totM), dtype=np.float32)
    off = 0
    for (koff, K, mat) in packs:
        arr[koff:koff + K, off:off + mat.shape[1]] = mat.astype(np.float32)
        off += mat.shape[1]
    return arr


def kernel(input, wavelet):
    from concourse.bass_utils import run_bass_kernel_spmd

    input = np.ascontiguousarray(np.asarray(input, dtype=np.float32))
    wavelet = np.asarray(wavelet, dtype=np.float32)
    plan = build_plan(wavelet)

    key = "prog"
    if key not in _CACHE:
        _CACHE[key] = _build_bass(plan)
    nc, packs, mpacks, totM, totN = _CACHE[key]

    mats = _pack_values(packs, totM)
    movs = _pack_values(mpacks, totN)
    ident = np.eye(128, dtype=np.float32)

    in_maps = []
    for c in range(NCORES):
        in_maps.append({
            "xin": input[ROWS_PER_CORE * c:ROWS_PER_CORE * (c + 1)],
            "mats": mats, "movs": movs, "ident": ident,
        })
    res = run_bass_kernel_spmd(nc, in_maps, list(range(NCORES)))
    rec = np.concatenate([res.results[c]["rec_out"] for c in range(NCORES)], axis=0)
    coef = np.concatenate([res.results[c]["coef_out"] for c in range(NCORES)], axis=0)
    return rec, coef


# revision 9
# speedup vs baseline: 1.4769x; 1.4769x over previous
"""Trainium2 Bass kernel for nn_Despawn2D (8-level row-wise DWT fwd+inv).

Self-contained. Host builds small banded operator matrices from the 8x16
wavelet (data-independent preprocessing); the 4096x4096 transform runs as
TensorE matmuls on 8 NeuronCores, data-parallel over rows (512 rows/core).

Layouts:
  - intermediates column-major: tiles [<=128 cols (partitions), 512 rows (free)]
  - fwd/inv levels: operator-stationary matmuls, out_block += mat.T @ in_block
    (1-2 full-K mains + one K=32 wrap slice at partition 96 per output block)
  - final rec + coeffs emitted row-major via data-stationary matmuls
    (stationary = data block, moving = narrow banded operator / selector)
"""
import os
import numpy as np
from collections import defaultdict

NCOLS = 4096
NROWS = 4096
LEVELS = 8
BW = 128
NCORES = 8
ROWS_PER_CORE = NROWS // NCORES  # 512
RCHUNKS = ROWS_PER_CORE // BW    # 4
CH = 512                         # rec output chunk width
USE_F32R = True
PAIRW = int(os.environ.get("KPAIRW", "2"))

# ------------------------------------------------------------------ plan

def tile_width(name):
    if name == "x":
        return NCOLS
    if name[0] in "da":
        return NCOLS >> (int(name[1]) + 1)
    if name == "c5":
        return 128
    if name == "c6":
        return 64
    if name == "c7":
        return 32
    if name.startswith("rec"):
        return NCOLS >> int(name[3])
    raise KeyError(name)


def coeff_col_off(l):
    return sum(NCOLS >> (i + 1) for i in range(l))


def _filters(w):
    alt = (-1.0) ** np.arange(16)
    g = w[::-1] * alt
    return w[0::2], w[1::2], g[0::2], g[1::2], w[::-1], w * alt


def _fwd_input_view(l):
    if l == 0:
        return "x", 0
    if 1 <= l <= 5:
        return f"a{l-1}", 0
    if l == 6:
        return "c5", 64
    return "c6", 32


def _fwd_out_spec(l, t, part):
    if l <= 4:
        return (f"d{l}", t), (f"a{l}", t)
    name = {5: "c5", 6: "c6", 7: "c7"}[l]
    return (name, t), (name, part + t)


def _circular_span(cols, n):
    cols = np.sort(cols)
    gaps = np.diff(np.concatenate([cols, [cols[0] + n]]))
    gi = int(np.argmax(gaps))
    return int(cols[(gi + 1) % cols.size]), int(n - gaps[gi] + 1)


def build_plan(wavelet):
    """Descriptors with (value matrix, structural matrix). Structure (slicing,
    dedup keys) is wavelet-independent via the structural mask."""
    wavelet = np.asarray(wavelet, dtype=np.float64)
    wstruct = np.arange(1.0, 17.0)[None, :] * (1.0 + np.arange(LEVELS)[:, None])
    plan = {"fwd": [], "inv": [], "inv0": [], "extract": []}

    def fwd_level(l):
        we, wo, ge, go, _, _ = _filters(wavelet[l])
        s_we, s_wo, s_ge, s_go, _, _ = _filters(wstruct[l])
        L = NCOLS >> l
        part = L // 2
        in_tile, in_off = _fwd_input_view(l)
        acc = defaultdict(lambda: defaultdict(lambda: np.zeros((2, BW, BW))))
        for t in range(part):
            (dt_t, dt_c), (at_t, at_c) = _fwd_out_spec(l, t, part)
            for k in range(8):
                s = (t - k) % part
                for j, cd, ca, sd, sa in ((2 * s, we[k], ge[k], s_we[k], s_ge[k]),
                                          (2 * s + 1, wo[k], go[k], s_wo[k], s_go[k])):
                    p = in_off + j
                    ib, ir = p // BW, p % BW
                    a1 = acc[(dt_t, dt_c // BW)][ib]
                    a1[0, ir, dt_c % BW] += cd
                    a1[1, ir, dt_c % BW] += sd
                    a2 = acc[(at_t, at_c // BW)][ib]
                    a2[0, ir, at_c % BW] += ca
                    a2[1, ir, at_c % BW] += sa
        return _emit(acc, in_tile)

    for l in range(LEVELS):
        plan["fwd"].append(fwd_level(l))

    for l in range(LEVELS - 1, 0, -1):
        _, _, _, _, ws, ss = _filters(wavelet[l])
        _, _, _, _, s_ws, s_ss = _filters(wstruct[l])
        P = NCOLS >> (l + 1)
        n2 = 2 * P
        d_tile = f"d{l}" if l <= 4 else {5: "c5", 6: "c6", 7: "c7"}[l]
        r_tile, r_off = ("c7", 16) if l == 7 else (f"rec{l+1}", 0)
        out_tile = f"rec{l}"
        accs = {d_tile: defaultdict(lambda: defaultdict(lambda: np.zeros((2, BW, BW)))),
                r_tile: defaultdict(lambda: defaultdict(lambda: np.zeros((2, BW, BW))))}
        for t in range(n2):
            ob, oc = t // BW, t % BW
            for m in range(16):
                u = (t - m) % n2
                if u % 2 == 0:
                    s = u // 2
                    ad = accs[d_tile][(out_tile, ob)][s // BW]
                    ad[0, s % BW, oc] += ws[m]
                    ad[1, s % BW, oc] += s_ws[m]
                    pr = r_off + s
                    ar = accs[r_tile][(out_tile, ob)][pr // BW]
                    ar[0, pr % BW, oc] += ss[m]
                    ar[1, pr % BW, oc] += s_ss[m]
        descs = []
        for tname, a in accs.items():
            descs += _emit(a, tname)
        plan["inv"].append((l, descs))

    # inverse level 0 -> row-major rec via data-stationary matmuls
    _, _, _, _, ws, ss = _filters(wavelet[0])
    _, _, _, _, s_ws, s_ss = _filters(wstruct[0])
    P = NCOLS // 2
    mov = {"d0": defaultdict(lambda: np.zeros((2, BW, NCOLS))),
           "rec1": defaultdict(lambda: np.zeros((2, BW, NCOLS)))}
    for t in range(NCOLS):
        for m in range(16):
            u = (t - m) % NCOLS
            if u % 2 == 0:
                s = u // 2
                md = mov["d0"][s // BW]
                md[0, s % BW, t] += ws[m]
                md[1, s % BW, t] += s_ws[m]
                mr = mov["rec1"][s // BW]
                mr[0, s % BW, t] += ss[m]
                mr[1, s % BW, t] += s_ss[m]
    for tname in ("d0", "rec1"):
        for blk in range(P // BW):
            full = mov[tname][blk]
            cols = np.nonzero(np.any(full[1] != 0.0, axis=0))[0]
            lo, width = _circular_span(cols, NCOLS)
            pos = 0
            while pos < width:
                g = (lo + pos) % NCOLS
                take = min(width - pos, CH - (g % CH))
                idx = (g + np.arange(take)) % NCOLS
                sub, subs = full[0][:, idx], full[1][:, idx]
                rows = np.nonzero(np.any(subs != 0.0, axis=1))[0]
                if rows.min() >= 96:
                    plan["inv0"].append((tname, blk, 96, 32, sub[96:128].copy(),
                                         subs[96:128].copy(), g // CH, g % CH, take))
                else:
                    plan["inv0"].append((tname, blk, 0, BW, sub.copy(), subs.copy(),
                                         g // CH, g % CH, take))
                pos += take

    # extraction -> row-major coeffs (selectors are constant matrices)
    ext = plan["extract"]
    for l in range(5):
        width = NCOLS >> (l + 1)
        off = coeff_col_off(l)
        for blk in range(width // BW):
            ext.append((f"d{l}", blk, BW, np.eye(BW), off + BW * blk, BW))
    sel = np.zeros((128, 64)); sel[:64, :] = np.eye(64)
    ext.append(("c5", 0, 128, sel, coeff_col_off(5), 64))
    sel6 = np.zeros((64, 32)); sel6[:32, :] = np.eye(32)
    ext.append(("c6", 0, 64, sel6, coeff_col_off(6), 32))
    ext.append(("c7", 0, 32, np.eye(32), coeff_col_off(7), 32))
    return plan


def _emit(acc, in_tile):
    descs = []
    inw = tile_width(in_tile)
    for (out_tile, ob), inblocks in sorted(acc.items()):
        obw = min(BW, tile_width(out_tile))
        for ib, mat in sorted(inblocks.items()):
            ibw = min(BW, inw - ib * BW)
            m, ms = mat[0, :ibw, :obw], mat[1, :ibw, :obw]
            rows = np.nonzero(np.any(ms != 0.0, axis=1))[0]
            if rows.size == 0:
                continue
            if ibw == BW and rows.min() >= 96:
                descs.append((out_tile, ob, obw, in_tile, ib, 96, 32,
                              m[96:128].copy(), ms[96:128].copy()))
            else:
                descs.append((out_tile, ob, obw, in_tile, ib, 0, ibw,
                              m.copy(), ms.copy()))
    return descs


# ---------------------------------------------------------------- packing

def pack_plan(plan):
    """Walk the plan canonically; dedupe matrices by STRUCTURAL content so
    offsets are wavelet-independent. Returns packed arrays + flat descriptors."""
    mat_off, packs, totM = {}, [], 0
    mov_off, mpacks, totN = {}, [], 0

    def pack_mat(koff, K, mat, smat):
        nonlocal totM
        key = (koff, K, smat.shape[1], smat.tobytes())
        if key not in mat_off:
            mat_off[key] = totM
            packs.append((koff, K, mat))
            totM += smat.shape[1]
        return mat_off[key]

    def pack_mov(koff, K, mat, smat):
        nonlocal totN
        key = (koff, K, smat.shape[1], smat.tobytes())
        if key not in mov_off:
            mov_off[key] = totN
            mpacks.append((koff, K, mat))
            totN += smat.shape[1]
        return mov_off[key]

    col_descs = []
    for l in range(LEVELS):
        for (ot, ob, obw, it, ib, koff, K, mat, smat) in plan["fwd"][l]:
            col_descs.append((ot, ob, obw, it, ib, koff, K,
                              pack_mat(koff, K, mat, smat), mat.shape[1]))
    for l, descs in plan["inv"]:
        for (ot, ob, obw, it, ib, koff, K, mat, smat) in descs:
            col_descs.append((ot, ob, obw, it, ib, koff, K,
                              pack_mat(koff, K, mat, smat), mat.shape[1]))
    inv0_descs = []
    for (st, blk, koff, K, mat, smat, ch, noff, N) in plan["inv0"]:
        inv0_descs.append((st, blk, koff, K, pack_mov(koff, K, mat, smat), ch, noff, N))
    ext_descs = []
    for (st, blk, K, mat, coff, N) in plan["extract"]:
        ext_descs.append((st, blk, K, pack_mov(0, K, mat, mat), coff, N))

    mats_arr = np.zeros((128, totM), dtype=np.float32)
    off = 0
    for (koff, K, mat) in packs:
        mats_arr[koff:koff + K, off:off + mat.shape[1]] = mat
        off += mat.shape[1]
    movs_arr = np.zeros((128, totN), dtype=np.float32)
    off = 0
    for (koff, K, mat) in mpacks:
        movs_arr[koff:koff + K, off:off + mat.shape[1]] = mat
        off += mat.shape[1]
    return mats_arr, movs_arr, col_descs, inv0_descs, ext_descs


# ------------------------------------------------------------- bass build

def _build_bass(col_descs, inv0_descs, ext_descs, totM, totN):
    import concourse.bacc as bacc
    import concourse.mybir as mybir
    from concourse.tile import TileContext

    F32 = mybir.dt.float32
    DT = mybir.dt.float32r if USE_F32R else F32

    nc = bacc.Bacc(None, target_bir_lowering=False)
    xin = nc.declare_dram_parameter("xin", [ROWS_PER_CORE, NCOLS], DT, isOutput=False)
    mats_d = nc.declare_dram_parameter("mats", [128, totM], DT, isOutput=False)
    movs_d = nc.declare_dram_parameter("movs", [128, totN], DT, isOutput=False)
    ident_d = nc.declare_dram_parameter("ident", [128, 128], DT, isOutput=False)
    rec_out = nc.declare_dram_parameter("rec_out", [ROWS_PER_CORE, NCOLS], F32,
                                        isOutput=True)
    coef_out = nc.declare_dram_parameter("coef_out", [ROWS_PER_CORE, NCOLS], F32,
                                         isOutput=True)

    with TileContext(nc) as tc:
        with tc.tile_pool(name="const", bufs=1) as cpool, \
             tc.tile_pool(name="xrm", bufs=8) as xrm_pool, \
             tc.tile_pool(name="tiles", bufs=52) as tpool, \
             tc.tile_pool(name="stg", bufs=2) as stg_pool, \
             tc.tile_pool(name="ps", bufs=1, space="PSUM") as psp:

            mats_sb = cpool.tile([128, totM], DT, tag="mats")
            movs_sb = cpool.tile([128, totN], DT, tag="movs")
            ident_sb = cpool.tile([128, 128], DT, tag="ident")
            nc.sync.dma_start(out=mats_sb[:], in_=mats_d[:])
            nc.sync.dma_start(out=movs_sb[:], in_=movs_d[:])
            nc.sync.dma_start(out=ident_sb[:], in_=ident_d[:])

            evac_ctr = [0]

            def evac(dst_ap, src_ap):
                if evac_ctr[0] % 2 == 0:
                    nc.scalar.copy(dst_ap, src_ap)
                else:
                    nc.vector.tensor_copy(dst_ap, src_ap)
                evac_ctr[0] += 1

            tiles = {}

            def get_tile(name, blk):
                key = (name, blk)
                if key not in tiles:
                    tiles[key] = tpool.tile([BW, ROWS_PER_CORE], DT, tag="t",
                                            name=f"{name}_{blk}")
                return tiles[key]

            def get_in_ap(name, blk, koff, K):
                return get_tile(name, blk)[koff:koff + K, :]

            def get_stat_ap(name, blk, koff, K, r):
                return get_tile(name, blk)[koff:koff + K, BW * r:BW * (r + 1)]

            psum_ctr = [0]

            def psum_tile():
                t = psp.tile([128, CH], F32, tag=f"pp{psum_ctr[0] % 6}",
                             name=f"ps{psum_ctr[0]}")
                psum_ctr[0] += 1
                return t

            # ---- input transposes: block-major, one [128,512] evac per block
            for h in range(4):
                xrs = []
                for r in range(RCHUNKS):
                    xr = xrm_pool.tile([BW, NCOLS // 4], DT, tag="xrm",
                                       name=f"xrm{h}_{r}")
                    nc.sync.dma_start(out=xr[:], in_=xin[BW * r:BW * (r + 1),
                                                         1024 * h:1024 * (h + 1)])
                    xrs.append(xr)
                for bb in range(8):
                    b = 8 * h + bb
                    pt = psum_tile()
                    for r in range(RCHUNKS):
                        nc.tensor.matmul(pt[:, BW * r:BW * (r + 1)],
                                         xrs[r][:, BW * bb:BW * (bb + 1)],
                                         ident_sb[:], start=True, stop=True)
                    evac(get_tile("x", b)[:], pt[:])

            # ---- extraction batches (emitted early, per source level) ----
            ext_batches = []
            cur, curw = [], 0
            for d in sorted(ext_descs, key=lambda d: d[4]):
                if curw + d[5] > CH:
                    ext_batches.append(cur)
                    cur, curw = [], 0
                cur.append((d, curw))
                curw += d[5]
            if cur:
                ext_batches.append(cur)

            def src_level(tile_name):
                if tile_name[0] == "d":
                    return int(tile_name[1])
                return {"c5": 5, "c6": 6, "c7": 7}[tile_name]

            def emit_ext_after(lvl_done):
                # emit batches whose every piece's source level is <= lvl_done
                # and which haven't been emitted yet
                for bi, batch in enumerate(ext_batches):
                    if bi in ext_emitted:
                        continue
                    if max(src_level(d[0]) for d, _ in batch) > lvl_done:
                        continue
                    ext_emitted.add(bi)
                    for r in range(RCHUNKS):
                        pt = psum_tile()
                        w = 0
                        for (st, blk, K, moff, coff, N), po in batch:
                            lhsT = get_stat_ap(st, blk, 0, K, r)
                            rhs = movs_sb[0:K, moff:moff + N]
                            nc.tensor.matmul(pt[:, po:po + N], lhsT, rhs,
                                             start=True, stop=True)
                            w = po + N
                        coff0 = batch[0][0][4]
                        stg = stg_pool.tile([BW, CH], F32, tag="cstg",
                                            name=f"cs{bi}_{r}")
                        evac(stg[:, 0:w], pt[:, 0:w])
                        nc.sync.dma_start(
                            out=coef_out[BW * r:BW * (r + 1), coff0:coff0 + w],
                            in_=stg[:, 0:w])

            ext_emitted = set()

            # ---- column-major levels; pairwise-interleaved accumulate groups
            groups, order = defaultdict(list), []
            for d in col_descs:
                key = (d[0], d[1])
                if key not in groups:
                    order.append(key)
                groups[key].append(d)

            def out_level(ot):
                if ot[0] == "d" or ot[0] == "a":
                    return ("fwd", int(ot[1]))
                if ot[0] == "c":
                    return ("fwd", int(ot[1]))
                return ("inv", int(ot[3]))

            def emit_col_pairs(keys):
                gi = 0
                while gi < len(keys):
                    pair = keys[gi:gi + PAIRW]
                    gi += len(pair)
                    pts = [psum_tile() for _ in pair]
                    seqs = []
                    for k, p in zip(pair, pts):
                        descs = groups[k]
                        seqs.append([(p, mats_sb[koff:koff + K, moff:moff + M],
                                      get_in_ap(it, ib, koff, K), i == 0,
                                      i == len(descs) - 1,
                                      (96, 0) if koff == 96 else None, M)
                                     for i, (_, _, _, it, ib, koff, K, moff, M)
                                     in enumerate(descs)])
                    mlen = max(len(s) for s in seqs)
                    for i in range(mlen):
                        for s in seqs:
                            if i < len(s):
                                p, lhsT, rhs, st, sp, tp, M = s[i]
                                nc.tensor.matmul(p[0:M, :], lhsT, rhs, start=st,
                                                 stop=sp, tile_position=tp)
                    for k, p in zip(pair, pts):
                        ot, ob = k
                        obw = groups[k][0][2]
                        evac(get_tile(ot, ob)[0:obw, :], p[0:obw, :])

            # forward levels with early extraction
            fwd_keys = defaultdict(list)
            inv_keys = []
            for key in order:
                kind, lvl = out_level(key[0])
                if kind == "fwd":
                    fwd_keys[lvl].append(key)
                else:
                    inv_keys.append(key)
            for l in range(LEVELS):
                emit_col_pairs(fwd_keys[l])
                emit_ext_after(l)
            emit_col_pairs(inv_keys)

            # ---- inv0 (rec, row-major), chunk pairs interleaved ----
            by_chunk = defaultdict(list)
            for d in inv0_descs:
                by_chunk[d[5]].append(d)
            for r in range(RCHUNKS):
                for chp in range(0, NCOLS // CH, 2):
                    stg = stg_pool.tile([BW, 1024], F32, tag="rstg",
                                        name=f"rstg{r}_{chp}")
                    pts = [psum_tile(), psum_tile()]
                    seqs = []
                    for j, ch in enumerate((chp, chp + 1)):
                        dl = by_chunk[ch]
                        seqs.append([(pts[j],
                                      get_stat_ap(st, blk, koff, K, r),
                                      movs_sb[koff:koff + K, moff:moff + N],
                                      i == 0, i == len(dl) - 1,
                                      (96, 0) if koff == 96 else None, noff, N)
                                     for i, (st, blk, koff, K, moff, _, noff, N)
                                     in enumerate(dl)])
                    mlen = max(len(s) for s in seqs)
                    for i in range(mlen):
                        for s in seqs:
                            if i < len(s):
                                p, lhsT, rhs, st_, sp, tp, noff, N = s[i]
                                nc.tensor.matmul(p[:, noff:noff + N], lhsT, rhs,
                                                 start=st_, stop=sp,
                                                 tile_position=tp)
                    for j in range(2):
                        evac(stg[:, CH * j:CH * (j + 1)], pts[j][:])
                    nc.sync.dma_start(
                        out=rec_out[BW * r:BW * (r + 1),
                                    1024 * (chp // 2):1024 * (chp // 2) + 1024],
                        in_=stg[:])

    nc.compile()
    return nc


# ---------------------------------------------------------------- runtime
_CACHE = {}


def kernel(input, wavelet):
    from concourse.bass_utils import run_bass_kernel_spmd

    input = np.ascontiguousarray(np.asarray(input, dtype=np.float32))
    wavelet = np.asarray(wavelet, dtype=np.float32)
    plan = build_plan(wavelet)
    mats, movs, col_descs, inv0_descs, ext_descs = pack_plan(plan)

    if "nc" not in _CACHE:
        _CACHE["nc"] = _build_bass(col_descs, inv0_descs, ext_descs,
                                   mats.shape[1], movs.shape[1])
    nc = _CACHE["nc"]

    ident = np.eye(128, dtype=np.float32)
    in_maps = []
    for c in range(NCORES):
        in_maps.append({
            "xin": input[ROWS_PER_CORE * c:ROWS_PER_CORE * (c + 1)],
            "mats": mats, "movs": movs, "ident": ident,
        })
    res = run_bass_kernel_spmd(nc, in_maps, list(range(NCORES)))
    rec = np.concatenate([res.results[c]["rec_out"] for c in range(NCORES)], axis=0)
    coef = np.concatenate([res.results[c]["coef_out"] for c in range(NCORES)], axis=0)
    return rec, coef
